# revision 21
# baseline (speedup 1.0000x reference)
"""DependencySelfAttention kernel for 8 Trainium2 NeuronCores.

Math (per batch b, reference semantics):
    sim  = P @ P^T                       [S, S]
    x    = sim * m[None, :]              (m = premise_mask as float)
    E    = exp(x - c[:, None])           c = row stabilizer (see below)
    Z    = sum_j E                       (softmax denominator)
    D1   = sum_{j: m=1} E + 1e-13 * Z    (re-normalization denom, exact)
    out  = (E * m[None,:] * dep) @ P * (pm / D1)[:, None]

Device-side simplifications (all exact w.r.t. the reference math):
  * Pm = P * m[:, None] is used for BOTH matmul operands: rows with m[i]=0
    produce garbage that the final pm[i]=0 scale kills; columns with m[j]=0
    give x[:, j] = 0 exactly, which matches sim * m.
  * The dependency mask is folded in AFTER exp:  G = E * dep_bf16, and
    out = (G^T as lhsT) @ Pm with per-row scale pm/D1 applied at the end.
    Since G already carries the m[j] factor (via exp of masked sims times
    dep... dep*m zeroing), AV uses Pm so m[j] is redundant but harmless.
  * sum_{j: m=1} E = Z - n0 * exp(-c) where n0 = #{j: m[j]=0}, because every
    masked column contributes exp(0 - c) exactly.
  * c = max(diag_i, max over x[:, ::16], 0) is a valid softmax stabilizer:
    it is always an actually-attained value (or 0 = the masked columns'
    value), so exp(x - c) never overflows and the denominator keeps at
    least one O(1) term.

Sharding: pure data parallel, 2 batches per core across 8 cores.
"""

import sys
from contextlib import ExitStack

import numpy as np
import ml_dtypes

for _p in ("/opt/trn_rl_repo", "/root/.axon_site/_ro/trn_rl_repo"):
    if _p not in sys.path:
        sys.path.insert(0, _p)

import bass_rust  # noqa: E402
from concourse import bass, bass_utils, mybir, tile  # noqa: E402

F32 = mybir.dt.float32
BF16 = mybir.dt.bfloat16
F16 = mybir.dt.float16
ALU = mybir.AluOpType
ACTF = mybir.ActivationFunctionType

B_FULL, S_FULL, D_FULL = 16, 2048, 256
N_CORES = 8


def split_multi_waits(nc, max_waits=1):
    """The walrus in this container encodes at most one sync-wait command per
    instruction. Hoist extra waits onto single-wait NoOps on the same engine
    right before the instruction — semantically identical (same blocking
    point in the engine's program order)."""
    n_split = 0
    counter = 0
    for func in nc.m.functions:
        for block in func.blocks:
            new = []
            for inst in list(block.instructions):
                si = getattr(inst, "sync_info", None)
                ow = list(si.on_wait) if si is not None and si.on_wait else []
                if len(ow) > max_waits:
                    for w in ow[:-max_waits]:
                        counter += 1
                        nop = bass_rust.InstNoOp(
                            name=f"WSPLIT-{counter}", ins=[], outs=[]
                        )
                        nop.engine = inst.engine
                        nop.sync_info = mybir.SyncInfo(on_wait=[w], on_update=[])
                        nop.debug = inst.debug
                        nc.register_instruction(nop, overwrite=True)
                        new.append(nop)
                    inst.sync_info = mybir.SyncInfo(
                        on_wait=ow[-max_waits:], on_update=list(si.on_update or [])
                    )
                    n_split += 1
                new.append(inst)
            block.instructions = new
    return n_split


def build_nc(BPC: int, S: int, D: int, rep: int = 1) -> bass.Bass:
    """Build the per-core Bass module. BPC batches, sequence S, feature D.
    rep > 1 repeats the whole computation (for slope-based timing)."""
    NB = S // 128          # number of 128-row blocks
    KC = D // 128          # contraction chunks for QK (128 each)
    QN = min(S, 512)       # QK moving free dim (fp32 max 512)
    NQ = S // QN           # QK chunks per row-block
    SUB = 16               # stabilizer subsample stride

    nc = bass.Bass("TRN2", target_bir_lowering=False, debug=False)

    pmt_d = nc.dram_tensor("pmt", [BPC, KC, 128, S], F32, kind="ExternalInput").ap()
    pmbf_d = nc.dram_tensor("pmbf", [BPC, 128, NB * D], F16, kind="ExternalInput").ap()
    dep_d = nc.dram_tensor("dep", [BPC, S, S], F16, kind="ExternalInput").ap()
    negc_d = nc.dram_tensor("negc", [BPC, 128, NB], F32, kind="ExternalInput").ap()
    n0e_d = nc.dram_tensor("n0e", [BPC, 128, NB], F32, kind="ExternalInput").ap()
    pmv_d = nc.dram_tensor("pmv", [BPC, 128, NB], F32, kind="ExternalInput").ap()
    out_d = nc.dram_tensor("out", [BPC, S, D], F32, kind="ExternalOutput").ap()

    with ExitStack() as ctx:
        tc = ctx.enter_context(tile.TileContext(nc))
        const = ctx.enter_context(tc.tile_pool(name="const", bufs=1))
        batchp = ctx.enter_context(tc.tile_pool(name="batch", bufs=2))
        depp = ctx.enter_context(tc.tile_pool(name="depp", bufs=3))
        ework = ctx.enter_context(tc.tile_pool(name="ework", bufs=3))
        gwork = ctx.enter_context(tc.tile_pool(name="gwork", bufs=3))
        outp = ctx.enter_context(tc.tile_pool(name="outp", bufs=3))
        smalls = ctx.enter_context(tc.tile_pool(name="smalls", bufs=6))
        xp = ctx.enter_context(tc.tile_pool(name="xp", bufs=1, space="PSUM"))
        up = ctx.enter_context(tc.tile_pool(name="up", bufs=2, space="PSUM"))

        def emit_av(st):
            """Lagged second matmul + output scale/store for a finished block."""
            GT_, s_, pmbf_, b_, ib_ = st
            U = up.tile([128, D], F32, tag="U")
            for jc in range(NB):
                nc.tensor.matmul(
                    U[:],
                    lhsT=GT_[:, jc * 128:(jc + 1) * 128],
                    rhs=pmbf_[:, jc * D:(jc + 1) * D],
                    start=(jc == 0),
                    stop=(jc == NB - 1),
                )
            osb = outp.tile([128, D], F32, tag="osb")
            nc.vector.tensor_scalar_mul(osb[:], U[:], s_[:])
            nc.sync.dma_start(out_d[b_, ib_ * 128:(ib_ + 1) * 128, :], osb[:])

        pending = []
        for b in [bb for _ in range(rep) for bb in range(BPC)]:
            pmt_sb = batchp.tile([128, KC * S], F32, tag="pmt")
            for kc in range(KC):
                nc.sync.dma_start(pmt_sb[:, kc * S:(kc + 1) * S], pmt_d[b, kc])
            pmbf_sb = batchp.tile([128, NB * D], F16, tag="pmbf")
            nc.sync.dma_start(pmbf_sb[:], pmbf_d[b])
            negc_sb = batchp.tile([128, NB], F32, tag="negc")
            nc.sync.dma_start(negc_sb[:], negc_d[b])
            n0e_sb = batchp.tile([128, NB], F32, tag="n0e")
            nc.sync.dma_start(n0e_sb[:], n0e_d[b])
            pmv_sb = batchp.tile([128, NB], F32, tag="pmv")
            nc.sync.dma_start(pmv_sb[:], pmv_d[b])

            for ib in range(NB):
                dep_sb = depp.tile([128, S], F16, tag="dep")
                nc.sync.dma_start(dep_sb[:], dep_d[b, ib * 128:(ib + 1) * 128, :])

                # x[i, j] = Pm_i . Pm_j  for this 128-row block, all S columns
                x = xp.tile([128, S], F32, tag="x")
                for kc in range(KC):
                    for jc in range(NQ):
                        nc.tensor.matmul(
                            x[:, jc * QN:(jc + 1) * QN],
                            lhsT=pmt_sb[:, kc * S + ib * 128: kc * S + ib * 128 + 128],
                            rhs=pmt_sb[:, kc * S + jc * QN: kc * S + (jc + 1) * QN],
                            start=(kc == 0),
                            stop=(kc == KC - 1),
                            skip_group_check=True,
                        )

                # E = exp(x - c), Z = row sum of E (fused); bias from host
                E = ework.tile([128, S], F32, tag="E")
                Z = smalls.tile([128, 1], F32, tag="Z")
                nc.scalar.activation(
                    E[:], x[:], ACTF.Exp, bias=negc_sb[:, ib:ib + 1],
                    scale=1.0, accum_out=Z[:]
                )

                # D1 = Z - n0*exp(-c)  (the 1e-13*Z term is below f32 ulp)
                d1 = smalls.tile([128, 1], F32, tag="d1")
                nc.vector.scalar_tensor_tensor(
                    d1[:], Z[:], 1.0 + 1e-13, n0e_sb[:, ib:ib + 1],
                    op0=ALU.mult, op1=ALU.subtract
                )
                rec = smalls.tile([128, 1], F32, tag="rec")
                nc.vector.reciprocal(rec[:], d1[:])
                s = smalls.tile([128, 1], F32, tag="s")
                nc.vector.tensor_scalar_mul(s[:], rec[:], pmv_sb[:, ib:ib + 1])

                # G = E * dep: halves on Pool and DVE in parallel
                G = gwork.tile([128, S], F16, tag="G")
                h = S // 2
                nc.gpsimd.tensor_tensor(G[:, 0:h], E[:, 0:h], dep_sb[:, 0:h],
                                        op=ALU.mult)
                nc.vector.tensor_tensor(G[:, h:S], E[:, h:S], dep_sb[:, h:S],
                                        op=ALU.mult)

                # GT[j, i] = G[i, j]: all NB 128x128 tiles transposed in one
                # xbar DMA (out AP places transposed blocks side by side)
                GT = gwork.tile([128, S], F16, tag="GT")
                gt_view = GT[:].rearrange("p (jc i) -> p jc i", i=128)
                nc.sync.dma_start(gt_view, G[:], transpose=True)

                # AV lags two blocks: keeps PE busy while this block's
                # softmax chain (exp -> G -> GT transpose) completes off-PE.
                pending.append((GT, s, pmbf_sb, b, ib))
                if len(pending) > 2:
                    emit_av(pending.pop(0))

        for st in pending:
            emit_av(st)

    split_multi_waits(nc)
    return nc


def make_in_maps(premise_batch, premise_mask, dependency_mask, n_cores=N_CORES):
    """Host-side preprocessing + sharding. Returns (in_maps, BPC)."""
    B, S, D = premise_batch.shape
    BPC = B // n_cores
    NB = S // 128
    KC = D // 128

    m = np.asarray(premise_mask).astype(np.float32)                    # [B,S]
    Pm = np.asarray(premise_batch).astype(np.float32) * m[:, :, None]  # [B,S,D]
    PmT = np.ascontiguousarray(Pm.transpose(0, 2, 1)).reshape(B, KC, 128, S)
    Pm_bf = np.ascontiguousarray(
        Pm.astype(np.float16).reshape(B, NB, 128, D).transpose(0, 2, 1, 3)
    ).reshape(B, 128, NB * D)
    dep_bf = np.asarray(dependency_mask).astype(np.float16)    # [B,S,S]
    diag = np.einsum("bsd,bsd->bs", Pm, Pm).astype(np.float32)
    negc = -np.maximum(diag, 0.0).astype(np.float32)                   # [B,S]
    n0 = (S - m.sum(axis=1)).astype(np.float32)                        # [B]
    n0e = (n0[:, None] * np.exp(negc)).astype(np.float32)              # [B,S]
    negc_r = np.ascontiguousarray(negc.reshape(B, NB, 128).transpose(0, 2, 1))
    n0e_r = np.ascontiguousarray(n0e.reshape(B, NB, 128).transpose(0, 2, 1))
    pmv_r = np.ascontiguousarray(m.reshape(B, NB, 128).transpose(0, 2, 1))

    in_maps = []
    for k in range(n_cores):
        sl = slice(k * BPC, (k + 1) * BPC)
        in_maps.append(
            {
                "pmt": np.ascontiguousarray(PmT[sl]),
                "pmbf": np.ascontiguousarray(Pm_bf[sl]),
                "dep": np.ascontiguousarray(dep_bf[sl]),
                "negc": np.ascontiguousarray(negc_r[sl]),
                "n0e": np.ascontiguousarray(n0e_r[sl]),
                "pmv": np.ascontiguousarray(pmv_r[sl]),
            }
        )
    return in_maps, BPC


_CACHE: dict = {}


def _built_nc() -> bass.Bass:
    if "nc" not in _CACHE:
        _CACHE["nc"] = build_nc(B_FULL // N_CORES, S_FULL, D_FULL)
    return _CACHE["nc"]


def kernel(premise_batch, premise_mask, dependency_mask, **run_kwargs):
    in_maps, _ = make_in_maps(premise_batch, premise_mask, dependency_mask)
    nc = _built_nc()
    res = bass_utils.run_bass_kernel_spmd(
        nc, in_maps, list(range(N_CORES)), **run_kwargs
    )
    outs = [np.asarray(res.results[k]["out"]) for k in range(N_CORES)]
    full = np.concatenate(outs, axis=0).astype(np.float32)
    if run_kwargs:
        _CACHE["last_results"] = res
    return full


# revision 22
# speedup vs baseline: 1.0316x; 1.0316x over previous
"""DependencySelfAttention kernel for 8 Trainium2 NeuronCores.

Math (per batch b, reference semantics):
    sim  = P @ P^T                       [S, S]
    x    = sim * m[None, :]              (m = premise_mask as float)
    E    = exp(x - c[:, None])           c = row stabilizer (see below)
    Z    = sum_j E                       (softmax denominator)
    D1   = sum_{j: m=1} E + 1e-13 * Z    (re-normalization denom, exact)
    out  = (E * m[None,:] * dep) @ P * (pm / D1)[:, None]

Device-side simplifications (all exact w.r.t. the reference math):
  * Pm = P * m[:, None] is used for BOTH matmul operands: rows with m[i]=0
    produce garbage that the final pm[i]=0 scale kills; columns with m[j]=0
    give x[:, j] = 0 exactly, which matches sim * m.
  * The dependency mask is folded in AFTER exp:  G = E * dep_bf16, and
    out = (G^T as lhsT) @ Pm with per-row scale pm/D1 applied at the end.
    Since G already carries the m[j] factor (via exp of masked sims times
    dep... dep*m zeroing), AV uses Pm so m[j] is redundant but harmless.
  * sum_{j: m=1} E = Z - n0 * exp(-c) where n0 = #{j: m[j]=0}, because every
    masked column contributes exp(0 - c) exactly.
  * c = max(diag_i, max over x[:, ::16], 0) is a valid softmax stabilizer:
    it is always an actually-attained value (or 0 = the masked columns'
    value), so exp(x - c) never overflows and the denominator keeps at
    least one O(1) term.

Sharding: pure data parallel, 2 batches per core across 8 cores.
"""

import sys
from contextlib import ExitStack

import numpy as np
import ml_dtypes

for _p in ("/opt/trn_rl_repo", "/root/.axon_site/_ro/trn_rl_repo"):
    if _p not in sys.path:
        sys.path.insert(0, _p)

import bass_rust  # noqa: E402
from concourse import bass, bass_utils, mybir, tile  # noqa: E402

F32 = mybir.dt.float32
BF16 = mybir.dt.bfloat16
F16 = mybir.dt.float16
ALU = mybir.AluOpType
ACTF = mybir.ActivationFunctionType

B_FULL, S_FULL, D_FULL = 16, 2048, 256
N_CORES = 8


def split_multi_waits(nc, max_waits=1):
    """The walrus in this container encodes at most one sync-wait command per
    instruction. Hoist extra waits onto single-wait NoOps on the same engine
    right before the instruction — semantically identical (same blocking
    point in the engine's program order)."""
    n_split = 0
    counter = 0
    for func in nc.m.functions:
        for block in func.blocks:
            new = []
            for inst in list(block.instructions):
                si = getattr(inst, "sync_info", None)
                ow = list(si.on_wait) if si is not None and si.on_wait else []
                if len(ow) > max_waits:
                    for w in ow[:-max_waits]:
                        counter += 1
                        nop = bass_rust.InstNoOp(
                            name=f"WSPLIT-{counter}", ins=[], outs=[]
                        )
                        nop.engine = inst.engine
                        nop.sync_info = mybir.SyncInfo(on_wait=[w], on_update=[])
                        nop.debug = inst.debug
                        nc.register_instruction(nop, overwrite=True)
                        new.append(nop)
                    inst.sync_info = mybir.SyncInfo(
                        on_wait=ow[-max_waits:], on_update=list(si.on_update or [])
                    )
                    n_split += 1
                new.append(inst)
            block.instructions = new
    return n_split


def build_nc(BPC: int, S: int, D: int, rep: int = 1) -> bass.Bass:
    """Build the per-core Bass module. BPC batches, sequence S, feature D.
    rep > 1 repeats the whole computation (for slope-based timing)."""
    NB = S // 128          # number of 128-row blocks
    KC = D // 128          # contraction chunks for QK (128 each)
    QN = min(S, 512)       # QK moving free dim (fp32 max 512)
    NQ = S // QN           # QK chunks per row-block
    SUB = 16               # stabilizer subsample stride

    nc = bass.Bass("TRN2", target_bir_lowering=False, debug=False)

    pmt_d = nc.dram_tensor("pmt", [BPC, KC, 128, S], F16, kind="ExternalInput").ap()
    pmbf_d = nc.dram_tensor("pmbf", [BPC, 128, NB * D], F16, kind="ExternalInput").ap()
    dep_d = nc.dram_tensor("dep", [BPC, S, S], F16, kind="ExternalInput").ap()
    negc_d = nc.dram_tensor("negc", [BPC, 128, NB], F32, kind="ExternalInput").ap()
    n0e_d = nc.dram_tensor("n0e", [BPC, 128, NB], F32, kind="ExternalInput").ap()
    pmv_d = nc.dram_tensor("pmv", [BPC, 128, NB], F32, kind="ExternalInput").ap()
    out_d = nc.dram_tensor("out", [BPC, S, D], F32, kind="ExternalOutput").ap()

    with ExitStack() as ctx:
        tc = ctx.enter_context(tile.TileContext(nc))
        const = ctx.enter_context(tc.tile_pool(name="const", bufs=1))
        batchp = ctx.enter_context(tc.tile_pool(name="batch", bufs=2))
        depp = ctx.enter_context(tc.tile_pool(name="depp", bufs=3))
        ework = ctx.enter_context(tc.tile_pool(name="ework", bufs=3))
        gwork = ctx.enter_context(tc.tile_pool(name="gwork", bufs=3))
        outp = ctx.enter_context(tc.tile_pool(name="outp", bufs=3))
        smalls = ctx.enter_context(tc.tile_pool(name="smalls", bufs=6))
        xp = ctx.enter_context(tc.tile_pool(name="xp", bufs=1, space="PSUM"))
        up = ctx.enter_context(tc.tile_pool(name="up", bufs=2, space="PSUM"))

        def emit_av(st):
            """Lagged second matmul + output scale/store for a finished block."""
            GT_, s_, pmbf_, b_, ib_ = st
            U = up.tile([128, D], F32, tag="U")
            for jc in range(NB):
                nc.tensor.matmul(
                    U[:],
                    lhsT=GT_[:, jc * 128:(jc + 1) * 128],
                    rhs=pmbf_[:, jc * D:(jc + 1) * D],
                    start=(jc == 0),
                    stop=(jc == NB - 1),
                )
            osb = outp.tile([128, D], F32, tag="osb")
            nc.vector.tensor_scalar_mul(osb[:], U[:], s_[:])
            nc.sync.dma_start(out_d[b_, ib_ * 128:(ib_ + 1) * 128, :], osb[:])

        pending = []
        for b in [bb for _ in range(rep) for bb in range(BPC)]:
            pmt_sb = batchp.tile([128, KC * S], F16, tag="pmt")
            for kc in range(KC):
                nc.sync.dma_start(pmt_sb[:, kc * S:(kc + 1) * S], pmt_d[b, kc])
            pmbf_sb = batchp.tile([128, NB * D], F16, tag="pmbf")
            nc.sync.dma_start(pmbf_sb[:], pmbf_d[b])
            negc_sb = batchp.tile([128, NB], F32, tag="negc")
            nc.sync.dma_start(negc_sb[:], negc_d[b])
            n0e_sb = batchp.tile([128, NB], F32, tag="n0e")
            nc.sync.dma_start(n0e_sb[:], n0e_d[b])
            pmv_sb = batchp.tile([128, NB], F32, tag="pmv")
            nc.sync.dma_start(pmv_sb[:], pmv_d[b])

            for ib in range(NB):
                dep_sb = depp.tile([128, S], F16, tag="dep")
                nc.sync.dma_start(dep_sb[:], dep_d[b, ib * 128:(ib + 1) * 128, :])

                # x[i, j] = Pm_i . Pm_j  for this 128-row block, all S columns
                x = xp.tile([128, S], F32, tag="x")
                for kc in range(KC):
                    for jc in range(NQ):
                        nc.tensor.matmul(
                            x[:, jc * QN:(jc + 1) * QN],
                            lhsT=pmt_sb[:, kc * S + ib * 128: kc * S + ib * 128 + 128],
                            rhs=pmt_sb[:, kc * S + jc * QN: kc * S + (jc + 1) * QN],
                            start=(kc == 0),
                            stop=(kc == KC - 1),
                            skip_group_check=True,
                        )

                # E = exp(x - c), Z = row sum of E (fused); bias from host
                E = ework.tile([128, S], F32, tag="E")
                Z = smalls.tile([128, 1], F32, tag="Z")
                nc.scalar.activation(
                    E[:], x[:], ACTF.Exp, bias=negc_sb[:, ib:ib + 1],
                    scale=1.0, accum_out=Z[:]
                )

                # D1 = Z - n0*exp(-c)  (the 1e-13*Z term is below f32 ulp)
                d1 = smalls.tile([128, 1], F32, tag="d1")
                nc.vector.scalar_tensor_tensor(
                    d1[:], Z[:], 1.0 + 1e-13, n0e_sb[:, ib:ib + 1],
                    op0=ALU.mult, op1=ALU.subtract
                )
                rec = smalls.tile([128, 1], F32, tag="rec")
                nc.vector.reciprocal(rec[:], d1[:])
                s = smalls.tile([128, 1], F32, tag="s")
                nc.vector.tensor_scalar_mul(s[:], rec[:], pmv_sb[:, ib:ib + 1])

                # G = E * dep: halves on Pool and DVE in parallel
                G = gwork.tile([128, S], F16, tag="G")
                h = S // 2
                nc.gpsimd.tensor_tensor(G[:, 0:h], E[:, 0:h], dep_sb[:, 0:h],
                                        op=ALU.mult)
                nc.vector.tensor_tensor(G[:, h:S], E[:, h:S], dep_sb[:, h:S],
                                        op=ALU.mult)

                # GT[j, i] = G[i, j]: all NB 128x128 tiles transposed in one
                # xbar DMA (out AP places transposed blocks side by side)
                GT = gwork.tile([128, S], F16, tag="GT")
                gt_view = GT[:].rearrange("p (jc i) -> p jc i", i=128)
                nc.sync.dma_start(gt_view, G[:], transpose=True)

                # AV lags two blocks: keeps PE busy while this block's
                # softmax chain (exp -> G -> GT transpose) completes off-PE.
                pending.append((GT, s, pmbf_sb, b, ib))
                if len(pending) > 2:
                    emit_av(pending.pop(0))

        for st in pending:
            emit_av(st)

    split_multi_waits(nc)
    return nc


def make_in_maps(premise_batch, premise_mask, dependency_mask, n_cores=N_CORES):
    """Host-side preprocessing + sharding. Returns (in_maps, BPC)."""
    B, S, D = premise_batch.shape
    BPC = B // n_cores
    NB = S // 128
    KC = D // 128

    m = np.asarray(premise_mask).astype(np.float32)                    # [B,S]
    Pm = np.asarray(premise_batch).astype(np.float32) * m[:, :, None]  # [B,S,D]
    PmT = np.ascontiguousarray(Pm.transpose(0, 2, 1)).astype(np.float16).reshape(B, KC, 128, S)
    Pm_bf = np.ascontiguousarray(
        Pm.astype(np.float16).reshape(B, NB, 128, D).transpose(0, 2, 1, 3)
    ).reshape(B, 128, NB * D)
    dep_bf = np.asarray(dependency_mask).astype(np.float16)    # [B,S,S]
    diag = np.einsum("bsd,bsd->bs", Pm, Pm).astype(np.float32)
    negc = -np.maximum(diag, 0.0).astype(np.float32)                   # [B,S]
    n0 = (S - m.sum(axis=1)).astype(np.float32)                        # [B]
    n0e = (n0[:, None] * np.exp(negc)).astype(np.float32)              # [B,S]
    negc_r = np.ascontiguousarray(negc.reshape(B, NB, 128).transpose(0, 2, 1))
    n0e_r = np.ascontiguousarray(n0e.reshape(B, NB, 128).transpose(0, 2, 1))
    pmv_r = np.ascontiguousarray(m.reshape(B, NB, 128).transpose(0, 2, 1))

    in_maps = []
    for k in range(n_cores):
        sl = slice(k * BPC, (k + 1) * BPC)
        in_maps.append(
            {
                "pmt": np.ascontiguousarray(PmT[sl]),
                "pmbf": np.ascontiguousarray(Pm_bf[sl]),
                "dep": np.ascontiguousarray(dep_bf[sl]),
                "negc": np.ascontiguousarray(negc_r[sl]),
                "n0e": np.ascontiguousarray(n0e_r[sl]),
                "pmv": np.ascontiguousarray(pmv_r[sl]),
            }
        )
    return in_maps, BPC


_CACHE: dict = {}


def _built_nc() -> bass.Bass:
    if "nc" not in _CACHE:
        _CACHE["nc"] = build_nc(B_FULL // N_CORES, S_FULL, D_FULL)
    return _CACHE["nc"]


def kernel(premise_batch, premise_mask, dependency_mask, **run_kwargs):
    in_maps, _ = make_in_maps(premise_batch, premise_mask, dependency_mask)
    nc = _built_nc()
    res = bass_utils.run_bass_kernel_spmd(
        nc, in_maps, list(range(N_CORES)), **run_kwargs
    )
    outs = [np.asarray(res.results[k]["out"]) for k in range(N_CORES)]
    full = np.concatenate(outs, axis=0).astype(np.float32)
    if run_kwargs:
        _CACHE["last_results"] = res
    return full


# revision 23
# speedup vs baseline: 1.1099x; 1.0759x over previous
"""DependencySelfAttention kernel for 8 Trainium2 NeuronCores.

Math (per batch b, reference semantics):
    sim  = P @ P^T                       [S, S]
    x    = sim * m[None, :]              (m = premise_mask as float)
    E    = exp(x - c[:, None])           c = row stabilizer (see below)
    Z    = sum_j E                       (softmax denominator)
    D1   = sum_{j: m=1} E + 1e-13 * Z    (re-normalization denom, exact)
    out  = (E * m[None,:] * dep) @ P * (pm / D1)[:, None]

Device-side simplifications (all exact w.r.t. the reference math):
  * Pm = P * m[:, None] is used for BOTH matmul operands: rows with m[i]=0
    produce garbage that the final pm[i]=0 scale kills; columns with m[j]=0
    give x[:, j] = 0 exactly, which matches sim * m.
  * The dependency mask is folded in AFTER exp:  G = E * dep_bf16, and
    out = (G^T as lhsT) @ Pm with per-row scale pm/D1 applied at the end.
    Since G already carries the m[j] factor (via exp of masked sims times
    dep... dep*m zeroing), AV uses Pm so m[j] is redundant but harmless.
  * sum_{j: m=1} E = Z - n0 * exp(-c) where n0 = #{j: m[j]=0}, because every
    masked column contributes exp(0 - c) exactly.
  * c = max(diag_i, max over x[:, ::16], 0) is a valid softmax stabilizer:
    it is always an actually-attained value (or 0 = the masked columns'
    value), so exp(x - c) never overflows and the denominator keeps at
    least one O(1) term.

Sharding: pure data parallel, 2 batches per core across 8 cores.
"""

import sys
from contextlib import ExitStack

import numpy as np
import ml_dtypes

for _p in ("/opt/trn_rl_repo", "/root/.axon_site/_ro/trn_rl_repo"):
    if _p not in sys.path:
        sys.path.insert(0, _p)

import bass_rust  # noqa: E402
from concourse import bass, bass_utils, mybir, tile  # noqa: E402

F32 = mybir.dt.float32
BF16 = mybir.dt.bfloat16
F16 = mybir.dt.float16
ALU = mybir.AluOpType
ACTF = mybir.ActivationFunctionType

B_FULL, S_FULL, D_FULL = 16, 2048, 256
N_CORES = 8


def split_multi_waits(nc, max_waits=1):
    """The walrus in this container encodes at most one sync-wait command per
    instruction. Hoist extra waits onto single-wait NoOps on the same engine
    right before the instruction — semantically identical (same blocking
    point in the engine's program order)."""
    n_split = 0
    counter = 0
    for func in nc.m.functions:
        for block in func.blocks:
            new = []
            for inst in list(block.instructions):
                si = getattr(inst, "sync_info", None)
                ow = list(si.on_wait) if si is not None and si.on_wait else []
                if len(ow) > max_waits:
                    for w in ow[:-max_waits]:
                        counter += 1
                        nop = bass_rust.InstNoOp(
                            name=f"WSPLIT-{counter}", ins=[], outs=[]
                        )
                        nop.engine = inst.engine
                        nop.sync_info = mybir.SyncInfo(on_wait=[w], on_update=[])
                        nop.debug = inst.debug
                        nc.register_instruction(nop, overwrite=True)
                        new.append(nop)
                    inst.sync_info = mybir.SyncInfo(
                        on_wait=ow[-max_waits:], on_update=list(si.on_update or [])
                    )
                    n_split += 1
                new.append(inst)
            block.instructions = new
    return n_split


def build_nc(BPC: int, S: int, D: int, rep: int = 1) -> bass.Bass:
    """Build the per-core Bass module. BPC batches, sequence S, feature D.
    rep > 1 repeats the whole computation (for slope-based timing)."""
    NB = S // 128          # number of 128-row blocks
    KC = D // 128          # contraction chunks for QK (128 each)
    QN = min(S, 512)       # QK moving free dim (fp32 max 512)
    NQ = S // QN           # QK chunks per row-block
    SUB = 16               # stabilizer subsample stride

    nc = bass.Bass("TRN2", target_bir_lowering=False, debug=False)

    pmt_d = nc.dram_tensor("pmt", [BPC, KC, 128, S], F16, kind="ExternalInput").ap()
    pmbf_d = nc.dram_tensor("pmbf", [BPC, 128, NB * D], F16, kind="ExternalInput").ap()
    dep_d = nc.dram_tensor("dep", [BPC, S, S], F16, kind="ExternalInput").ap()
    negc_d = nc.dram_tensor("negc", [BPC, 128, NB], F32, kind="ExternalInput").ap()
    n0e_d = nc.dram_tensor("n0e", [BPC, 128, NB], F32, kind="ExternalInput").ap()
    pmv_d = nc.dram_tensor("pmv", [BPC, 128, NB], F32, kind="ExternalInput").ap()
    out_d = nc.dram_tensor("out", [BPC, S, D], F32, kind="ExternalOutput").ap()

    with ExitStack() as ctx:
        tc = ctx.enter_context(tile.TileContext(nc))
        const = ctx.enter_context(tc.tile_pool(name="const", bufs=1))
        batchp = ctx.enter_context(tc.tile_pool(name="batch", bufs=2))
        depp = ctx.enter_context(tc.tile_pool(name="depp", bufs=3))
        ework = ctx.enter_context(tc.tile_pool(name="ework", bufs=3))
        gwork = ctx.enter_context(tc.tile_pool(name="gwork", bufs=3))
        outp = ctx.enter_context(tc.tile_pool(name="outp", bufs=3))
        smalls = ctx.enter_context(tc.tile_pool(name="smalls", bufs=6))
        xp = ctx.enter_context(tc.tile_pool(name="xp", bufs=1, space="PSUM"))
        up = ctx.enter_context(tc.tile_pool(name="up", bufs=2, space="PSUM"))

        def emit_av(st):
            """Lagged second matmul + output scale/store for a finished block."""
            GT_, s_, pmbf_, b_, ib_ = st
            U = up.tile([128, D], F32, tag="U")
            for jc in range(NB):
                nc.tensor.matmul(
                    U[:],
                    lhsT=GT_[:, jc * 128:(jc + 1) * 128],
                    rhs=pmbf_[:, jc * D:(jc + 1) * D],
                    start=(jc == 0),
                    stop=(jc == NB - 1),
                )
            osb = outp.tile([128, D], F32, tag="osb")
            nc.vector.tensor_scalar_mul(osb[:], U[:], s_[:])
            nc.sync.dma_start(out_d[b_, ib_ * 128:(ib_ + 1) * 128, :], osb[:])

        pending = []
        for b in [bb for _ in range(rep) for bb in range(BPC)]:
            pmt_sb = batchp.tile([128, KC * S], F16, tag="pmt")
            for kc in range(KC):
                nc.sync.dma_start(pmt_sb[:, kc * S:(kc + 1) * S], pmt_d[b, kc])
            pmbf_sb = batchp.tile([128, NB * D], F16, tag="pmbf")
            nc.sync.dma_start(pmbf_sb[:], pmbf_d[b])
            negc_sb = batchp.tile([128, NB], F32, tag="negc")
            nc.sync.dma_start(negc_sb[:], negc_d[b])
            n0e_sb = batchp.tile([128, NB], F32, tag="n0e")
            nc.sync.dma_start(n0e_sb[:], n0e_d[b])
            pmv_sb = batchp.tile([128, NB], F32, tag="pmv")
            nc.sync.dma_start(pmv_sb[:], pmv_d[b])

            for ib in range(NB):
                dep_sb = depp.tile([128, S], F16, tag="dep")
                nc.sync.dma_start(dep_sb[:], dep_d[b, ib * 128:(ib + 1) * 128, :])

                # x[i, j] = Pm_i . Pm_j  for this 128-row block, all S columns
                x = xp.tile([128, S], F32, tag="x")
                for kc in range(KC):
                    for jc in range(NQ):
                        nc.tensor.matmul(
                            x[:, jc * QN:(jc + 1) * QN],
                            lhsT=pmt_sb[:, kc * S + ib * 128: kc * S + ib * 128 + 128],
                            rhs=pmt_sb[:, kc * S + jc * QN: kc * S + (jc + 1) * QN],
                            start=(kc == 0),
                            stop=(kc == KC - 1),
                            skip_group_check=True,
                        )

                # E = exp(x - c), Z = row sum of E (fused); bias from host
                E = ework.tile([128, S], F32, tag="E")
                Z = smalls.tile([128, 1], F32, tag="Z")
                nc.scalar.activation(
                    E[:], x[:], ACTF.Exp, bias=negc_sb[:, ib:ib + 1],
                    scale=1.0, accum_out=Z[:]
                )

                # D1 = Z - n0*exp(-c)  (the 1e-13*Z term is below f32 ulp)
                d1 = smalls.tile([128, 1], F32, tag="d1")
                nc.vector.scalar_tensor_tensor(
                    d1[:], Z[:], 1.0 + 1e-13, n0e_sb[:, ib:ib + 1],
                    op0=ALU.mult, op1=ALU.subtract
                )
                rec = smalls.tile([128, 1], F32, tag="rec")
                nc.vector.reciprocal(rec[:], d1[:])
                s = smalls.tile([128, 1], F32, tag="s")
                nc.vector.tensor_scalar_mul(s[:], rec[:], pmv_sb[:, ib:ib + 1])

                # G = E * dep: halves on Pool and DVE in parallel
                G = gwork.tile([128, S], F16, tag="G")
                h = S // 2
                nc.gpsimd.tensor_tensor(G[:, 0:h], E[:, 0:h], dep_sb[:, 0:h],
                                        op=ALU.mult)
                nc.vector.tensor_tensor(G[:, h:S], E[:, h:S], dep_sb[:, h:S],
                                        op=ALU.mult)

                # GT[j, i] = G[i, j]: all NB 128x128 tiles transposed in one
                # xbar DMA (out AP places transposed blocks side by side)
                GT = gwork.tile([128, S], F16, tag="GT")
                gt_view = GT[:].rearrange("p (jc i) -> p jc i", i=128)
                nc.scalar.dma_start(gt_view, G[:], transpose=True)

                # AV lags two blocks: keeps PE busy while this block's
                # softmax chain (exp -> G -> GT transpose) completes off-PE.
                pending.append((GT, s, pmbf_sb, b, ib))
                if len(pending) > 2:
                    emit_av(pending.pop(0))

        for st in pending:
            emit_av(st)

    split_multi_waits(nc)
    return nc


def make_in_maps(premise_batch, premise_mask, dependency_mask, n_cores=N_CORES):
    """Host-side preprocessing + sharding. Returns (in_maps, BPC)."""
    B, S, D = premise_batch.shape
    BPC = B // n_cores
    NB = S // 128
    KC = D // 128

    m = np.asarray(premise_mask).astype(np.float32)                    # [B,S]
    Pm = np.asarray(premise_batch).astype(np.float32) * m[:, :, None]  # [B,S,D]
    PmT = np.ascontiguousarray(Pm.transpose(0, 2, 1)).astype(np.float16).reshape(B, KC, 128, S)
    Pm_bf = np.ascontiguousarray(
        Pm.astype(np.float16).reshape(B, NB, 128, D).transpose(0, 2, 1, 3)
    ).reshape(B, 128, NB * D)
    dep_bf = np.asarray(dependency_mask).astype(np.float16)    # [B,S,S]
    diag = np.einsum("bsd,bsd->bs", Pm, Pm).astype(np.float32)
    negc = -np.maximum(diag, 0.0).astype(np.float32)                   # [B,S]
    n0 = (S - m.sum(axis=1)).astype(np.float32)                        # [B]
    n0e = (n0[:, None] * np.exp(negc)).astype(np.float32)              # [B,S]
    negc_r = np.ascontiguousarray(negc.reshape(B, NB, 128).transpose(0, 2, 1))
    n0e_r = np.ascontiguousarray(n0e.reshape(B, NB, 128).transpose(0, 2, 1))
    pmv_r = np.ascontiguousarray(m.reshape(B, NB, 128).transpose(0, 2, 1))

    in_maps = []
    for k in range(n_cores):
        sl = slice(k * BPC, (k + 1) * BPC)
        in_maps.append(
            {
                "pmt": np.ascontiguousarray(PmT[sl]),
                "pmbf": np.ascontiguousarray(Pm_bf[sl]),
                "dep": np.ascontiguousarray(dep_bf[sl]),
                "negc": np.ascontiguousarray(negc_r[sl]),
                "n0e": np.ascontiguousarray(n0e_r[sl]),
                "pmv": np.ascontiguousarray(pmv_r[sl]),
            }
        )
    return in_maps, BPC


_CACHE: dict = {}


def _built_nc() -> bass.Bass:
    if "nc" not in _CACHE:
        _CACHE["nc"] = build_nc(B_FULL // N_CORES, S_FULL, D_FULL)
    return _CACHE["nc"]


def kernel(premise_batch, premise_mask, dependency_mask, **run_kwargs):
    in_maps, _ = make_in_maps(premise_batch, premise_mask, dependency_mask)
    nc = _built_nc()
    res = bass_utils.run_bass_kernel_spmd(
        nc, in_maps, list(range(N_CORES)), **run_kwargs
    )
    outs = [np.asarray(res.results[k]["out"]) for k in range(N_CORES)]
    full = np.concatenate(outs, axis=0).astype(np.float32)
    if run_kwargs:
        _CACHE["last_results"] = res
    return full


# revision 24
# speedup vs baseline: 1.2727x; 1.1467x over previous
"""DependencySelfAttention kernel for 8 Trainium2 NeuronCores.

Math (per batch b, reference semantics):
    sim  = P @ P^T                       [S, S]
    x    = sim * m[None, :]              (m = premise_mask as float)
    E    = exp(x - c[:, None])           c = row stabilizer (see below)
    Z    = sum_j E                       (softmax denominator)
    D1   = sum_{j: m=1} E + 1e-13 * Z    (re-normalization denom, exact)
    out  = (E * m[None,:] * dep) @ P * (pm / D1)[:, None]

Device-side simplifications (all exact w.r.t. the reference math):
  * Pm = P * m[:, None] is used for BOTH matmul operands: rows with m[i]=0
    produce garbage that the final pm[i]=0 scale kills; columns with m[j]=0
    give x[:, j] = 0 exactly, which matches sim * m.
  * The dependency mask is folded in AFTER exp:  G = E * dep_bf16, and
    out = (G^T as lhsT) @ Pm with per-row scale pm/D1 applied at the end.
    Since G already carries the m[j] factor (via exp of masked sims times
    dep... dep*m zeroing), AV uses Pm so m[j] is redundant but harmless.
  * sum_{j: m=1} E = Z - n0 * exp(-c) where n0 = #{j: m[j]=0}, because every
    masked column contributes exp(0 - c) exactly.
  * c = max(diag_i, max over x[:, ::16], 0) is a valid softmax stabilizer:
    it is always an actually-attained value (or 0 = the masked columns'
    value), so exp(x - c) never overflows and the denominator keeps at
    least one O(1) term.

Sharding: pure data parallel, 2 batches per core across 8 cores.
"""

import sys
from contextlib import ExitStack

import numpy as np
import ml_dtypes

for _p in ("/opt/trn_rl_repo", "/root/.axon_site/_ro/trn_rl_repo"):
    if _p not in sys.path:
        sys.path.insert(0, _p)

import bass_rust  # noqa: E402
from concourse import bass, bass_utils, mybir, tile  # noqa: E402

F32 = mybir.dt.float32
BF16 = mybir.dt.bfloat16
F16 = mybir.dt.float16
ALU = mybir.AluOpType
ACTF = mybir.ActivationFunctionType

B_FULL, S_FULL, D_FULL = 16, 2048, 256
N_CORES = 8


def split_multi_waits(nc, max_waits=1):
    """The walrus in this container encodes at most one sync-wait command per
    instruction. Hoist extra waits onto single-wait NoOps on the same engine
    right before the instruction — semantically identical (same blocking
    point in the engine's program order)."""
    n_split = 0
    counter = 0
    for func in nc.m.functions:
        for block in func.blocks:
            new = []
            for inst in list(block.instructions):
                si = getattr(inst, "sync_info", None)
                ow = list(si.on_wait) if si is not None and si.on_wait else []
                if len(ow) > max_waits:
                    for w in ow[:-max_waits]:
                        counter += 1
                        nop = bass_rust.InstNoOp(
                            name=f"WSPLIT-{counter}", ins=[], outs=[]
                        )
                        nop.engine = inst.engine
                        nop.sync_info = mybir.SyncInfo(on_wait=[w], on_update=[])
                        nop.debug = inst.debug
                        nc.register_instruction(nop, overwrite=True)
                        new.append(nop)
                    inst.sync_info = mybir.SyncInfo(
                        on_wait=ow[-max_waits:], on_update=list(si.on_update or [])
                    )
                    n_split += 1
                new.append(inst)
            block.instructions = new
    return n_split


def build_nc(BPC: int, S: int, D: int, rep: int = 1, no_transpose: bool = False) -> bass.Bass:
    """Build the per-core Bass module. BPC batches, sequence S, feature D.
    rep > 1 repeats the whole computation (for slope-based timing)."""
    NB = S // 128          # number of 128-row blocks
    KC = D // 128          # contraction chunks for QK (128 each)
    QN = min(S, 512)       # QK moving free dim (fp32 max 512)
    NQ = S // QN           # QK chunks per row-block
    SUB = 16               # stabilizer subsample stride

    nc = bass.Bass("TRN2", target_bir_lowering=False, debug=False)

    pmt_d = nc.dram_tensor("pmt", [BPC, KC, 128, S], F16, kind="ExternalInput").ap()
    pmbf_d = nc.dram_tensor("pmbf", [BPC, 128, NB * D], F16, kind="ExternalInput").ap()
    dep_d = nc.dram_tensor("dep", [BPC, S, S], F16, kind="ExternalInput").ap()
    negc_d = nc.dram_tensor("negc", [BPC, 128, NB], F32, kind="ExternalInput").ap()
    n0e_d = nc.dram_tensor("n0e", [BPC, 128, NB], F32, kind="ExternalInput").ap()
    pmv_d = nc.dram_tensor("pmv", [BPC, 128, NB], F32, kind="ExternalInput").ap()
    out_d = nc.dram_tensor("out", [BPC, S, D], F32, kind="ExternalOutput").ap()

    with ExitStack() as ctx:
        tc = ctx.enter_context(tile.TileContext(nc))
        const = ctx.enter_context(tc.tile_pool(name="const", bufs=1))
        batchp = ctx.enter_context(tc.tile_pool(name="batch", bufs=2))
        depp = ctx.enter_context(tc.tile_pool(name="depp", bufs=3))
        ework = ctx.enter_context(tc.tile_pool(name="ework", bufs=3))
        gwork = ctx.enter_context(tc.tile_pool(name="gwork", bufs=3))
        outp = ctx.enter_context(tc.tile_pool(name="outp", bufs=3))
        smalls = ctx.enter_context(tc.tile_pool(name="smalls", bufs=6))
        xp = ctx.enter_context(tc.tile_pool(name="xp", bufs=1, space="PSUM"))
        up = ctx.enter_context(tc.tile_pool(name="up", bufs=2, space="PSUM"))

        def emit_av(st):
            """Lagged second matmul + output scale/store for a finished block."""
            GT_, s_, pmbf_, b_, ib_ = st
            U = up.tile([128, D], F32, tag="U")
            for jc in range(NB):
                nc.tensor.matmul(
                    U[:],
                    lhsT=GT_[:, jc * 128:(jc + 1) * 128],
                    rhs=pmbf_[:, jc * D:(jc + 1) * D],
                    start=(jc == 0),
                    stop=(jc == NB - 1),
                )
            osb = outp.tile([128, D], F32, tag="osb")
            nc.vector.tensor_scalar_mul(osb[:], U[:], s_[:])
            nc.sync.dma_start(out_d[b_, ib_ * 128:(ib_ + 1) * 128, :], osb[:])

        pending = []
        for b in [bb for _ in range(rep) for bb in range(BPC)]:
            pmt_sb = batchp.tile([128, KC * S], F16, tag="pmt")
            for kc in range(KC):
                nc.sync.dma_start(pmt_sb[:, kc * S:(kc + 1) * S], pmt_d[b, kc])
            pmbf_sb = batchp.tile([128, NB * D], F16, tag="pmbf")
            nc.sync.dma_start(pmbf_sb[:], pmbf_d[b])
            negc_sb = batchp.tile([128, NB], F32, tag="negc")
            nc.sync.dma_start(negc_sb[:], negc_d[b])
            n0e_sb = batchp.tile([128, NB], F32, tag="n0e")
            nc.sync.dma_start(n0e_sb[:], n0e_d[b])
            pmv_sb = batchp.tile([128, NB], F32, tag="pmv")
            nc.sync.dma_start(pmv_sb[:], pmv_d[b])

            for ib in range(NB):
                dep_sb = depp.tile([128, S], F16, tag="dep")
                nc.sync.dma_start(dep_sb[:], dep_d[b, ib * 128:(ib + 1) * 128, :])

                # x[i, j] = Pm_i . Pm_j  for this 128-row block, all S columns
                x = xp.tile([128, S], F32, tag="x")
                for kc in range(KC):
                    for jc in range(NQ):
                        nc.tensor.matmul(
                            x[:, jc * QN:(jc + 1) * QN],
                            lhsT=pmt_sb[:, kc * S + ib * 128: kc * S + ib * 128 + 128],
                            rhs=pmt_sb[:, kc * S + jc * QN: kc * S + (jc + 1) * QN],
                            start=(kc == 0),
                            stop=(kc == KC - 1),
                            skip_group_check=True,
                        )

                # E = exp(x - c), Z = row sum of E (fused); bias from host
                E = ework.tile([128, S], F32, tag="E")
                Z = smalls.tile([128, 1], F32, tag="Z")
                nc.scalar.activation(
                    E[:], x[:], ACTF.Exp, bias=negc_sb[:, ib:ib + 1],
                    scale=1.0, accum_out=Z[:]
                )

                # D1 = Z - n0*exp(-c)  (the 1e-13*Z term is below f32 ulp)
                d1 = smalls.tile([128, 1], F32, tag="d1")
                nc.vector.scalar_tensor_tensor(
                    d1[:], Z[:], 1.0 + 1e-13, n0e_sb[:, ib:ib + 1],
                    op0=ALU.mult, op1=ALU.subtract
                )
                rec = smalls.tile([128, 1], F32, tag="rec")
                nc.vector.reciprocal(rec[:], d1[:])
                s = smalls.tile([128, 1], F32, tag="s")
                nc.vector.tensor_scalar_mul(s[:], rec[:], pmv_sb[:, ib:ib + 1])

                # G = E * dep: halves on Pool and DVE in parallel
                G = gwork.tile([128, S], F16, tag="G")
                h = S // 2
                nc.gpsimd.tensor_tensor(G[:, 0:h], E[:, 0:h], dep_sb[:, 0:h],
                                        op=ALU.mult)
                nc.vector.tensor_tensor(G[:, h:S], E[:, h:S], dep_sb[:, h:S],
                                        op=ALU.mult)

                # GT[j, i] = G[i, j]: all NB 128x128 tiles transposed in one
                # xbar DMA (out AP places transposed blocks side by side)
                GT = gwork.tile([128, S], F16, tag="GT")
                gt_view = GT[:].rearrange("p (jc i) -> p jc i", i=128)
                if no_transpose:
                    nc.scalar.dma_start(GT[:], G[:])  # timing probe only
                else:
                    nc.scalar.dma_start(gt_view, G[:], transpose=True)

                # AV lags two blocks: keeps PE busy while this block's
                # softmax chain (exp -> G -> GT transpose) completes off-PE.
                pending.append((GT, s, pmbf_sb, b, ib))
                if len(pending) > 2:
                    emit_av(pending.pop(0))

        for st in pending:
            emit_av(st)

    split_multi_waits(nc)
    return nc


def make_in_maps(premise_batch, premise_mask, dependency_mask, n_cores=N_CORES):
    """Host-side preprocessing + sharding. Returns (in_maps, BPC)."""
    B, S, D = premise_batch.shape
    BPC = B // n_cores
    NB = S // 128
    KC = D // 128

    m = np.asarray(premise_mask).astype(np.float32)                    # [B,S]
    Pm = np.asarray(premise_batch).astype(np.float32) * m[:, :, None]  # [B,S,D]
    PmT = np.ascontiguousarray(Pm.transpose(0, 2, 1)).astype(np.float16).reshape(B, KC, 128, S)
    Pm_bf = np.ascontiguousarray(
        Pm.astype(np.float16).reshape(B, NB, 128, D).transpose(0, 2, 1, 3)
    ).reshape(B, 128, NB * D)
    dep_bf = np.asarray(dependency_mask).astype(np.float16)    # [B,S,S]
    diag = np.einsum("bsd,bsd->bs", Pm, Pm).astype(np.float32)
    negc = -np.maximum(diag, 0.0).astype(np.float32)                   # [B,S]
    n0 = (S - m.sum(axis=1)).astype(np.float32)                        # [B]
    n0e = (n0[:, None] * np.exp(negc)).astype(np.float32)              # [B,S]
    negc_r = np.ascontiguousarray(negc.reshape(B, NB, 128).transpose(0, 2, 1))
    n0e_r = np.ascontiguousarray(n0e.reshape(B, NB, 128).transpose(0, 2, 1))
    pmv_r = np.ascontiguousarray(m.reshape(B, NB, 128).transpose(0, 2, 1))

    in_maps = []
    for k in range(n_cores):
        sl = slice(k * BPC, (k + 1) * BPC)
        in_maps.append(
            {
                "pmt": np.ascontiguousarray(PmT[sl]),
                "pmbf": np.ascontiguousarray(Pm_bf[sl]),
                "dep": np.ascontiguousarray(dep_bf[sl]),
                "negc": np.ascontiguousarray(negc_r[sl]),
                "n0e": np.ascontiguousarray(n0e_r[sl]),
                "pmv": np.ascontiguousarray(pmv_r[sl]),
            }
        )
    return in_maps, BPC


_CACHE: dict = {}


def _built_nc() -> bass.Bass:
    if "nc" not in _CACHE:
        _CACHE["nc"] = build_nc(B_FULL // N_CORES, S_FULL, D_FULL)
    return _CACHE["nc"]


def kernel(premise_batch, premise_mask, dependency_mask, **run_kwargs):
    in_maps, _ = make_in_maps(premise_batch, premise_mask, dependency_mask)
    nc = _built_nc()
    res = bass_utils.run_bass_kernel_spmd(
        nc, in_maps, list(range(N_CORES)), **run_kwargs
    )
    outs = [np.asarray(res.results[k]["out"]) for k in range(N_CORES)]
    full = np.concatenate(outs, axis=0).astype(np.float32)
    if run_kwargs:
        _CACHE["last_results"] = res
    return full


# revision 25
# speedup vs baseline: 1.3426x; 1.0549x over previous
"""DependencySelfAttention kernel for 8 Trainium2 NeuronCores.

Math (per batch b, reference semantics):
    sim  = P @ P^T                       [S, S]
    x    = sim * m[None, :]              (m = premise_mask as float)
    E    = exp(x - c[:, None])           c = row stabilizer (see below)
    Z    = sum_j E                       (softmax denominator)
    D1   = sum_{j: m=1} E + 1e-13 * Z    (re-normalization denom, exact)
    out  = (E * m[None,:] * dep) @ P * (pm / D1)[:, None]

Device-side simplifications (all exact w.r.t. the reference math):
  * Pm = P * m[:, None] is used for BOTH matmul operands: rows with m[i]=0
    produce garbage that the final pm[i]=0 scale kills; columns with m[j]=0
    give x[:, j] = 0 exactly, which matches sim * m.
  * The dependency mask is folded in AFTER exp:  G = E * dep_bf16, and
    out = (G^T as lhsT) @ Pm with per-row scale pm/D1 applied at the end.
    Since G already carries the m[j] factor (via exp of masked sims times
    dep... dep*m zeroing), AV uses Pm so m[j] is redundant but harmless.
  * sum_{j: m=1} E = Z - n0 * exp(-c) where n0 = #{j: m[j]=0}, because every
    masked column contributes exp(0 - c) exactly.
  * c = max(diag_i, max over x[:, ::16], 0) is a valid softmax stabilizer:
    it is always an actually-attained value (or 0 = the masked columns'
    value), so exp(x - c) never overflows and the denominator keeps at
    least one O(1) term.

Sharding: pure data parallel, 2 batches per core across 8 cores.
"""

import sys
from contextlib import ExitStack

import numpy as np
import ml_dtypes

for _p in ("/opt/trn_rl_repo", "/root/.axon_site/_ro/trn_rl_repo"):
    if _p not in sys.path:
        sys.path.insert(0, _p)

import bass_rust  # noqa: E402
from concourse import bass, bass_utils, mybir, tile  # noqa: E402

F32 = mybir.dt.float32
BF16 = mybir.dt.bfloat16
F16 = mybir.dt.float16
ALU = mybir.AluOpType
ACTF = mybir.ActivationFunctionType

B_FULL, S_FULL, D_FULL = 16, 2048, 256
N_CORES = 8


def split_multi_waits(nc, max_waits=1):
    """The walrus in this container encodes at most one sync-wait command per
    instruction. Hoist extra waits onto single-wait NoOps on the same engine
    right before the instruction — semantically identical (same blocking
    point in the engine's program order)."""
    n_split = 0
    counter = 0
    for func in nc.m.functions:
        for block in func.blocks:
            new = []
            for inst in list(block.instructions):
                si = getattr(inst, "sync_info", None)
                ow = list(si.on_wait) if si is not None and si.on_wait else []
                if len(ow) > max_waits:
                    for w in ow[:-max_waits]:
                        counter += 1
                        nop = bass_rust.InstNoOp(
                            name=f"WSPLIT-{counter}", ins=[], outs=[]
                        )
                        nop.engine = inst.engine
                        nop.sync_info = mybir.SyncInfo(on_wait=[w], on_update=[])
                        nop.debug = inst.debug
                        nc.register_instruction(nop, overwrite=True)
                        new.append(nop)
                    inst.sync_info = mybir.SyncInfo(
                        on_wait=ow[-max_waits:], on_update=list(si.on_update or [])
                    )
                    n_split += 1
                new.append(inst)
            block.instructions = new
    return n_split


def build_nc(BPC: int, S: int, D: int, rep: int = 1, no_transpose: bool = False, ablate: str = '') -> bass.Bass:
    """Build the per-core Bass module. BPC batches, sequence S, feature D.
    rep > 1 repeats the whole computation (for slope-based timing)."""
    NB = S // 128          # number of 128-row blocks
    KC = D // 128          # contraction chunks for QK (128 each)
    QN = min(S, 512)       # QK moving free dim (fp32 max 512)
    NQ = S // QN           # QK chunks per row-block
    SUB = 16               # stabilizer subsample stride

    nc = bass.Bass("TRN2", target_bir_lowering=False, debug=False)

    pmt_d = nc.dram_tensor("pmt", [BPC, KC, 128, S], F16, kind="ExternalInput").ap()
    pmbf_d = nc.dram_tensor("pmbf", [BPC, 128, NB * D], F16, kind="ExternalInput").ap()
    dep_d = nc.dram_tensor("dep", [BPC, S, S], F16, kind="ExternalInput").ap()
    negc_d = nc.dram_tensor("negc", [BPC, 128, NB], F32, kind="ExternalInput").ap()
    n0e_d = nc.dram_tensor("n0e", [BPC, 128, NB], F32, kind="ExternalInput").ap()
    pmv_d = nc.dram_tensor("pmv", [BPC, 128, NB], F32, kind="ExternalInput").ap()
    out_d = nc.dram_tensor("out", [BPC, S, D], F32, kind="ExternalOutput").ap()

    with ExitStack() as ctx:
        tc = ctx.enter_context(tile.TileContext(nc))
        const = ctx.enter_context(tc.tile_pool(name="const", bufs=1))
        batchp = ctx.enter_context(tc.tile_pool(name="batch", bufs=2))
        depp = ctx.enter_context(tc.tile_pool(name="depp", bufs=3))
        ework = ctx.enter_context(tc.tile_pool(name="ework", bufs=3))
        gwork = ctx.enter_context(tc.tile_pool(name="gwork", bufs=3))
        outp = ctx.enter_context(tc.tile_pool(name="outp", bufs=3))
        smalls = ctx.enter_context(tc.tile_pool(name="smalls", bufs=6))
        xp = ctx.enter_context(tc.tile_pool(name="xp", bufs=1, space="PSUM"))
        up = ctx.enter_context(tc.tile_pool(name="up", bufs=2, space="PSUM"))

        def emit_av(st):
            """Lagged second matmul + output scale/store for a finished block."""
            GT_, s_, pmbf_, b_, ib_ = st
            U = up.tile([128, D], F32, tag="U")
            for jc in range(NB):
                nc.tensor.matmul(
                    U[:],
                    lhsT=GT_[:, jc * 128:(jc + 1) * 128],
                    rhs=pmbf_[:, jc * D:(jc + 1) * D],
                    start=(jc == 0),
                    stop=(jc == NB - 1),
                )
            osb = outp.tile([128, D], F32, tag="osb")
            nc.vector.tensor_scalar_mul(osb[:], U[:], s_[:])
            nc.sync.dma_start(out_d[b_, ib_ * 128:(ib_ + 1) * 128, :], osb[:])

        pending = []
        for b in [bb for _ in range(rep) for bb in range(BPC)]:
            pmt_sb = batchp.tile([128, KC * S], F16, tag="pmt")
            for kc in range(KC):
                nc.sync.dma_start(pmt_sb[:, kc * S:(kc + 1) * S], pmt_d[b, kc])
            pmbf_sb = batchp.tile([128, NB * D], F16, tag="pmbf")
            nc.sync.dma_start(pmbf_sb[:], pmbf_d[b])
            negc_sb = batchp.tile([128, NB], F32, tag="negc")
            nc.sync.dma_start(negc_sb[:], negc_d[b])
            n0e_sb = batchp.tile([128, NB], F32, tag="n0e")
            nc.sync.dma_start(n0e_sb[:], n0e_d[b])
            pmv_sb = batchp.tile([128, NB], F32, tag="pmv")
            nc.sync.dma_start(pmv_sb[:], pmv_d[b])

            for ib in range(NB):
                dep_sb = depp.tile([128, S], F16, tag="dep")
                nc.sync.dma_start(dep_sb[:], dep_d[b, ib * 128:(ib + 1) * 128, :])

                # x[i, j] = Pm_i . Pm_j  for this 128-row block, all S columns
                x = xp.tile([128, S], F32, tag="x")
                for kc in range(KC):
                    for jc in range(NQ):
                        nc.tensor.matmul(
                            x[:, jc * QN:(jc + 1) * QN],
                            lhsT=pmt_sb[:, kc * S + ib * 128: kc * S + ib * 128 + 128],
                            rhs=pmt_sb[:, kc * S + jc * QN: kc * S + (jc + 1) * QN],
                            start=(kc == 0),
                            stop=(kc == KC - 1),
                            skip_group_check=True,
                        )

                # E = exp(x - c), Z = row sum of E (fused); bias from host
                E = ework.tile([128, S], F32, tag="E")
                Z = smalls.tile([128, 1], F32, tag="Z")
                nc.scalar.activation(
                    E[:], x[:], ACTF.Exp, bias=negc_sb[:, ib:ib + 1],
                    scale=1.0, accum_out=Z[:]
                )

                # D1 = Z - n0*exp(-c)  (the 1e-13*Z term is below f32 ulp)
                d1 = smalls.tile([128, 1], F32, tag="d1")
                nc.vector.scalar_tensor_tensor(
                    d1[:], Z[:], 1.0 + 1e-13, n0e_sb[:, ib:ib + 1],
                    op0=ALU.mult, op1=ALU.subtract
                )
                rec = smalls.tile([128, 1], F32, tag="rec")
                nc.vector.reciprocal(rec[:], d1[:])
                s = smalls.tile([128, 1], F32, tag="s")
                nc.vector.tensor_scalar_mul(s[:], rec[:], pmv_sb[:, ib:ib + 1])

                if ablate == "front":
                    # keep only QK/exp/stats + dep load + a tiny store
                    osb0 = outp.tile([128, D], F32, tag="osb")
                    nc.vector.tensor_scalar_mul(osb0[:], E[:, 0:D], s[:])
                    nc.sync.dma_start(out_d[b, ib * 128:(ib + 1) * 128, :], osb0[:])
                    continue
                # G = E * dep: halves on Pool and DVE in parallel
                G = gwork.tile([128, S], F16, tag="G")
                h = S // 2
                nc.gpsimd.tensor_tensor(G[:, 0:h], E[:, 0:h], dep_sb[:, 0:h],
                                        op=ALU.mult)
                nc.vector.tensor_tensor(G[:, h:S], E[:, h:S], dep_sb[:, h:S],
                                        op=ALU.mult)

                # GT[j, i] = G[i, j]: all NB 128x128 tiles transposed in one
                # xbar DMA (out AP places transposed blocks side by side)
                GT = gwork.tile([128, S], F16, tag="GT")
                gt_view = GT[:].rearrange("p (jc i) -> p jc i", i=128)
                if no_transpose:
                    nc.scalar.dma_start(GT[:], G[:])  # timing probe only
                else:
                    nc.scalar.dma_start(gt_view, G[:], transpose=True)

                # AV lags two blocks: keeps PE busy while this block's
                # softmax chain (exp -> G -> GT transpose) completes off-PE.
                pending.append((GT, s, pmbf_sb, b, ib))
                if len(pending) > 2:
                    emit_av(pending.pop(0))

        for st in pending:
            emit_av(st)
        del pending[:]

    split_multi_waits(nc)
    return nc


def make_in_maps(premise_batch, premise_mask, dependency_mask, n_cores=N_CORES):
    """Host-side preprocessing + sharding. Returns (in_maps, BPC)."""
    B, S, D = premise_batch.shape
    BPC = B // n_cores
    NB = S // 128
    KC = D // 128

    m = np.asarray(premise_mask).astype(np.float32)                    # [B,S]
    Pm = np.asarray(premise_batch).astype(np.float32) * m[:, :, None]  # [B,S,D]
    PmT = np.ascontiguousarray(Pm.transpose(0, 2, 1)).astype(np.float16).reshape(B, KC, 128, S)
    Pm_bf = np.ascontiguousarray(
        Pm.astype(np.float16).reshape(B, NB, 128, D).transpose(0, 2, 1, 3)
    ).reshape(B, 128, NB * D)
    dep_bf = np.asarray(dependency_mask).astype(np.float16)    # [B,S,S]
    diag = np.einsum("bsd,bsd->bs", Pm, Pm).astype(np.float32)
    negc = -np.maximum(diag, 0.0).astype(np.float32)                   # [B,S]
    n0 = (S - m.sum(axis=1)).astype(np.float32)                        # [B]
    n0e = (n0[:, None] * np.exp(negc)).astype(np.float32)              # [B,S]
    negc_r = np.ascontiguousarray(negc.reshape(B, NB, 128).transpose(0, 2, 1))
    n0e_r = np.ascontiguousarray(n0e.reshape(B, NB, 128).transpose(0, 2, 1))
    pmv_r = np.ascontiguousarray(m.reshape(B, NB, 128).transpose(0, 2, 1))

    in_maps = []
    for k in range(n_cores):
        sl = slice(k * BPC, (k + 1) * BPC)
        in_maps.append(
            {
                "pmt": np.ascontiguousarray(PmT[sl]),
                "pmbf": np.ascontiguousarray(Pm_bf[sl]),
                "dep": np.ascontiguousarray(dep_bf[sl]),
                "negc": np.ascontiguousarray(negc_r[sl]),
                "n0e": np.ascontiguousarray(n0e_r[sl]),
                "pmv": np.ascontiguousarray(pmv_r[sl]),
            }
        )
    return in_maps, BPC


_CACHE: dict = {}


def _built_nc() -> bass.Bass:
    if "nc" not in _CACHE:
        _CACHE["nc"] = build_nc(B_FULL // N_CORES, S_FULL, D_FULL)
    return _CACHE["nc"]


def kernel(premise_batch, premise_mask, dependency_mask, **run_kwargs):
    in_maps, _ = make_in_maps(premise_batch, premise_mask, dependency_mask)
    nc = _built_nc()
    res = bass_utils.run_bass_kernel_spmd(
        nc, in_maps, list(range(N_CORES)), **run_kwargs
    )
    outs = [np.asarray(res.results[k]["out"]) for k in range(N_CORES)]
    full = np.concatenate(outs, axis=0).astype(np.float32)
    if run_kwargs:
        _CACHE["last_results"] = res
    return full


# revision 26
# speedup vs baseline: 1.7256x; 1.2853x over previous
"""DependencySelfAttention kernel for 8 Trainium2 NeuronCores.

Math (per batch b, reference semantics):
    sim  = P @ P^T                       [S, S]
    x    = sim * m[None, :]              (m = premise_mask as float)
    E    = exp(x - c[:, None])           c = row stabilizer (see below)
    Z    = sum_j E                       (softmax denominator)
    D1   = sum_{j: m=1} E + 1e-13 * Z    (re-normalization denom, exact)
    out  = (E * m[None,:] * dep) @ P * (pm / D1)[:, None]

Device-side simplifications (all exact w.r.t. the reference math):
  * Pm = P * m[:, None] is used for BOTH matmul operands: rows with m[i]=0
    produce garbage that the final pm[i]=0 scale kills; columns with m[j]=0
    give x[:, j] = 0 exactly, which matches sim * m.
  * The dependency mask is folded in AFTER exp:  G = E * dep_bf16, and
    out = (G^T as lhsT) @ Pm with per-row scale pm/D1 applied at the end.
    Since G already carries the m[j] factor (via exp of masked sims times
    dep... dep*m zeroing), AV uses Pm so m[j] is redundant but harmless.
  * sum_{j: m=1} E = Z - n0 * exp(-c) where n0 = #{j: m[j]=0}, because every
    masked column contributes exp(0 - c) exactly.
  * c = max(diag_i, max over x[:, ::16], 0) is a valid softmax stabilizer:
    it is always an actually-attained value (or 0 = the masked columns'
    value), so exp(x - c) never overflows and the denominator keeps at
    least one O(1) term.

Sharding: pure data parallel, 2 batches per core across 8 cores.
"""

import sys
from contextlib import ExitStack

import numpy as np
import ml_dtypes

for _p in ("/opt/trn_rl_repo", "/root/.axon_site/_ro/trn_rl_repo"):
    if _p not in sys.path:
        sys.path.insert(0, _p)

import bass_rust  # noqa: E402
from concourse import bass, bass_utils, mybir, tile  # noqa: E402

F32 = mybir.dt.float32
BF16 = mybir.dt.bfloat16
F16 = mybir.dt.float16
ALU = mybir.AluOpType
ACTF = mybir.ActivationFunctionType

B_FULL, S_FULL, D_FULL = 16, 2048, 256
N_CORES = 8


def split_multi_waits(nc, max_waits=1):
    """The walrus in this container encodes at most one sync-wait command per
    instruction. Hoist extra waits onto single-wait NoOps on the same engine
    right before the instruction — semantically identical (same blocking
    point in the engine's program order)."""
    n_split = 0
    counter = 0
    for func in nc.m.functions:
        for block in func.blocks:
            new = []
            for inst in list(block.instructions):
                si = getattr(inst, "sync_info", None)
                ow = list(si.on_wait) if si is not None and si.on_wait else []
                if len(ow) > max_waits:
                    for w in ow[:-max_waits]:
                        counter += 1
                        nop = bass_rust.InstNoOp(
                            name=f"WSPLIT-{counter}", ins=[], outs=[]
                        )
                        nop.engine = inst.engine
                        nop.sync_info = mybir.SyncInfo(on_wait=[w], on_update=[])
                        nop.debug = inst.debug
                        nc.register_instruction(nop, overwrite=True)
                        new.append(nop)
                    inst.sync_info = mybir.SyncInfo(
                        on_wait=ow[-max_waits:], on_update=list(si.on_update or [])
                    )
                    n_split += 1
                new.append(inst)
            block.instructions = new
    return n_split


def build_nc(BPC: int, S: int, D: int, rep: int = 1, no_transpose: bool = False, ablate: str = '') -> bass.Bass:
    """Build the per-core Bass module. BPC batches, sequence S, feature D.
    rep > 1 repeats the whole computation (for slope-based timing)."""
    NB = S // 128          # number of 128-row blocks
    KC = D // 128          # contraction chunks for QK (128 each)
    QN = min(S, 512)       # QK moving free dim (fp32 max 512)
    NQ = S // QN           # QK chunks per row-block
    SUB = 16               # stabilizer subsample stride

    nc = bass.Bass("TRN2", target_bir_lowering=False, debug=False)

    pmt_d = nc.dram_tensor("pmt", [BPC, KC, 128, S], F16, kind="ExternalInput").ap()
    pmbf_d = nc.dram_tensor("pmbf", [BPC, 128, NB * D], F16, kind="ExternalInput").ap()
    dep_d = nc.dram_tensor("dep", [BPC, S, S], F16, kind="ExternalInput").ap()
    negc_d = nc.dram_tensor("negc", [BPC, 128, NB], F32, kind="ExternalInput").ap()
    n0e_d = nc.dram_tensor("n0e", [BPC, 128, NB], F32, kind="ExternalInput").ap()
    pmv_d = nc.dram_tensor("pmv", [BPC, 128, NB], F32, kind="ExternalInput").ap()
    out_d = nc.dram_tensor("out", [BPC, S, D], F32, kind="ExternalOutput").ap()

    with ExitStack() as ctx:
        tc = ctx.enter_context(tile.TileContext(nc))
        const = ctx.enter_context(tc.tile_pool(name="const", bufs=1))
        batchp = ctx.enter_context(tc.tile_pool(name="batch", bufs=2))
        depp = ctx.enter_context(tc.tile_pool(name="depp", bufs=3))
        ework = ctx.enter_context(tc.tile_pool(name="ework", bufs=3))
        gwork = ctx.enter_context(tc.tile_pool(name="gwork", bufs=3))
        outp = ctx.enter_context(tc.tile_pool(name="outp", bufs=3))
        smalls = ctx.enter_context(tc.tile_pool(name="smalls", bufs=6))
        xp = ctx.enter_context(tc.tile_pool(name="xp", bufs=1, space="PSUM"))
        up = ctx.enter_context(tc.tile_pool(name="up", bufs=2, space="PSUM"))

        def emit_av(st):
            """Lagged second matmul + output scale/store for a finished block."""
            GT_, s_, pmbf_, b_, ib_ = st
            U = up.tile([128, D], F32, tag="U")
            for jc in range(NB):
                nc.tensor.matmul(
                    U[:],
                    lhsT=GT_[:, jc * 128:(jc + 1) * 128],
                    rhs=pmbf_[:, jc * D:(jc + 1) * D],
                    start=(jc == 0),
                    stop=(jc == NB - 1),
                )
            osb = outp.tile([128, D], F32, tag="osb")
            nc.vector.tensor_scalar_mul(osb[:], U[:], s_[:])
            nc.sync.dma_start(out_d[b_, ib_ * 128:(ib_ + 1) * 128, :], osb[:])

        pending = []
        for b in [bb for _ in range(rep) for bb in range(BPC)]:
            pmt_sb = batchp.tile([128, KC * S], F16, tag="pmt")
            for kc in range(KC):
                nc.sync.dma_start(pmt_sb[:, kc * S:(kc + 1) * S], pmt_d[b, kc])
            pmbf_sb = batchp.tile([128, NB * D], F16, tag="pmbf")
            nc.sync.dma_start(pmbf_sb[:], pmbf_d[b])
            negc_sb = batchp.tile([128, NB], F32, tag="negc")
            nc.sync.dma_start(negc_sb[:], negc_d[b])
            n0e_sb = batchp.tile([128, NB], F32, tag="n0e")
            nc.sync.dma_start(n0e_sb[:], n0e_d[b])
            pmv_sb = batchp.tile([128, NB], F32, tag="pmv")
            nc.sync.dma_start(pmv_sb[:], pmv_d[b])

            for ib in range(NB):
                dep_sb = depp.tile([128, S], F16, tag="dep")
                nc.sync.dma_start(dep_sb[:], dep_d[b, ib * 128:(ib + 1) * 128, :])

                # x[i, j] = Pm_i . Pm_j  for this 128-row block, all S columns
                x = xp.tile([128, S], F32, tag="x")
                for kc in range(KC):
                    for jc in range(NQ):
                        nc.tensor.matmul(
                            x[:, jc * QN:(jc + 1) * QN],
                            lhsT=pmt_sb[:, kc * S + ib * 128: kc * S + ib * 128 + 128],
                            rhs=pmt_sb[:, kc * S + jc * QN: kc * S + (jc + 1) * QN],
                            start=(kc == 0),
                            stop=(kc == KC - 1),
                            skip_group_check=True,
                        )

                # E = exp(x - c), Z = row sum of E (fused); bias from host
                E = ework.tile([128, S], F32, tag="E")
                Z = smalls.tile([128, 1], F32, tag="Z")
                nc.scalar.activation(
                    E[:], x[:], ACTF.Exp, bias=negc_sb[:, ib:ib + 1],
                    scale=1.0, accum_out=Z[:]
                )

                # D1 = Z - n0*exp(-c)  (the 1e-13*Z term is below f32 ulp)
                d1 = smalls.tile([128, 1], F32, tag="d1")
                nc.vector.scalar_tensor_tensor(
                    d1[:], Z[:], 1.0 + 1e-13, n0e_sb[:, ib:ib + 1],
                    op0=ALU.mult, op1=ALU.subtract
                )
                rec = smalls.tile([128, 1], F32, tag="rec")
                nc.vector.reciprocal(rec[:], d1[:])
                s = smalls.tile([128, 1], F32, tag="s")
                nc.vector.tensor_scalar_mul(s[:], rec[:], pmv_sb[:, ib:ib + 1])

                if ablate == "qkonly":
                    xr = smalls.tile([128, 1], F32, tag="xr")
                    nc.vector.tensor_copy(xr[:], x[:, 0:1])
                    osb0 = outp.tile([128, D], F32, tag="osb")
                    nc.vector.tensor_scalar_mul(osb0[:], pmt_sb[:, 0:D], xr[:])
                    nc.sync.dma_start(out_d[b, ib * 128:(ib + 1) * 128, :], osb0[:])
                    continue
                if ablate == "front":
                    # keep only QK/exp/stats + dep load + a tiny store
                    osb0 = outp.tile([128, D], F32, tag="osb")
                    nc.vector.tensor_scalar_mul(osb0[:], E[:, 0:D], s[:])
                    nc.sync.dma_start(out_d[b, ib * 128:(ib + 1) * 128, :], osb0[:])
                    continue
                # G = E * dep: halves on Pool and DVE in parallel
                G = gwork.tile([128, S], F16, tag="G")
                h = S // 2
                nc.gpsimd.tensor_tensor(G[:, 0:h], E[:, 0:h], dep_sb[:, 0:h],
                                        op=ALU.mult)
                nc.vector.tensor_tensor(G[:, h:S], E[:, h:S], dep_sb[:, h:S],
                                        op=ALU.mult)

                # GT[j, i] = G[i, j]: all NB 128x128 tiles transposed in one
                # xbar DMA (out AP places transposed blocks side by side)
                GT = gwork.tile([128, S], F16, tag="GT")
                gt_view = GT[:].rearrange("p (jc i) -> p jc i", i=128)
                if no_transpose:
                    nc.scalar.dma_start(GT[:], G[:])  # timing probe only
                else:
                    nc.scalar.dma_start(gt_view, G[:], transpose=True)

                # AV lags two blocks: keeps PE busy while this block's
                # softmax chain (exp -> G -> GT transpose) completes off-PE.
                pending.append((GT, s, pmbf_sb, b, ib))
                if len(pending) > 2:
                    emit_av(pending.pop(0))

        for st in pending:
            emit_av(st)
        del pending[:]

    split_multi_waits(nc)
    return nc


def make_in_maps(premise_batch, premise_mask, dependency_mask, n_cores=N_CORES):
    """Host-side preprocessing + sharding. Returns (in_maps, BPC)."""
    B, S, D = premise_batch.shape
    BPC = B // n_cores
    NB = S // 128
    KC = D // 128

    m = np.asarray(premise_mask).astype(np.float32)                    # [B,S]
    Pm = np.asarray(premise_batch).astype(np.float32) * m[:, :, None]  # [B,S,D]
    PmT = np.ascontiguousarray(Pm.transpose(0, 2, 1)).astype(np.float16).reshape(B, KC, 128, S)
    Pm_bf = np.ascontiguousarray(
        Pm.astype(np.float16).reshape(B, NB, 128, D).transpose(0, 2, 1, 3)
    ).reshape(B, 128, NB * D)
    dep_bf = np.asarray(dependency_mask).astype(np.float16)    # [B,S,S]
    diag = np.einsum("bsd,bsd->bs", Pm, Pm).astype(np.float32)
    negc = -np.maximum(diag, 0.0).astype(np.float32)                   # [B,S]
    n0 = (S - m.sum(axis=1)).astype(np.float32)                        # [B]
    n0e = (n0[:, None] * np.exp(negc)).astype(np.float32)              # [B,S]
    negc_r = np.ascontiguousarray(negc.reshape(B, NB, 128).transpose(0, 2, 1))
    n0e_r = np.ascontiguousarray(n0e.reshape(B, NB, 128).transpose(0, 2, 1))
    pmv_r = np.ascontiguousarray(m.reshape(B, NB, 128).transpose(0, 2, 1))

    in_maps = []
    for k in range(n_cores):
        sl = slice(k * BPC, (k + 1) * BPC)
        in_maps.append(
            {
                "pmt": np.ascontiguousarray(PmT[sl]),
                "pmbf": np.ascontiguousarray(Pm_bf[sl]),
                "dep": np.ascontiguousarray(dep_bf[sl]),
                "negc": np.ascontiguousarray(negc_r[sl]),
                "n0e": np.ascontiguousarray(n0e_r[sl]),
                "pmv": np.ascontiguousarray(pmv_r[sl]),
            }
        )
    return in_maps, BPC


_CACHE: dict = {}


def _built_nc() -> bass.Bass:
    if "nc" not in _CACHE:
        _CACHE["nc"] = build_nc(B_FULL // N_CORES, S_FULL, D_FULL)
    return _CACHE["nc"]


def kernel(premise_batch, premise_mask, dependency_mask, **run_kwargs):
    in_maps, _ = make_in_maps(premise_batch, premise_mask, dependency_mask)
    nc = _built_nc()
    res = bass_utils.run_bass_kernel_spmd(
        nc, in_maps, list(range(N_CORES)), **run_kwargs
    )
    outs = [np.asarray(res.results[k]["out"]) for k in range(N_CORES)]
    full = np.concatenate(outs, axis=0).astype(np.float32)
    if run_kwargs:
        _CACHE["last_results"] = res
    return full


# revision 27
# speedup vs baseline: 1.9792x; 1.1470x over previous
"""DependencySelfAttention kernel for 8 Trainium2 NeuronCores.

Math (per batch b, reference semantics):
    sim  = P @ P^T                       [S, S]
    x    = sim * m[None, :]              (m = premise_mask as float)
    E    = exp(x - c[:, None])           c = row stabilizer (see below)
    Z    = sum_j E                       (softmax denominator)
    D1   = sum_{j: m=1} E + 1e-13 * Z    (re-normalization denom, exact)
    out  = (E * m[None,:] * dep) @ P * (pm / D1)[:, None]

Device-side simplifications (all exact w.r.t. the reference math):
  * Pm = P * m[:, None] is used for BOTH matmul operands: rows with m[i]=0
    produce garbage that the final pm[i]=0 scale kills; columns with m[j]=0
    give x[:, j] = 0 exactly, which matches sim * m.
  * The dependency mask is folded in AFTER exp:  G = E * dep_bf16, and
    out = (G^T as lhsT) @ Pm with per-row scale pm/D1 applied at the end.
    Since G already carries the m[j] factor (via exp of masked sims times
    dep... dep*m zeroing), AV uses Pm so m[j] is redundant but harmless.
  * sum_{j: m=1} E = Z - n0 * exp(-c) where n0 = #{j: m[j]=0}, because every
    masked column contributes exp(0 - c) exactly.
  * c = max(diag_i, max over x[:, ::16], 0) is a valid softmax stabilizer:
    it is always an actually-attained value (or 0 = the masked columns'
    value), so exp(x - c) never overflows and the denominator keeps at
    least one O(1) term.

Sharding: pure data parallel, 2 batches per core across 8 cores.
"""

import sys
from contextlib import ExitStack

import numpy as np
import ml_dtypes

for _p in ("/opt/trn_rl_repo", "/root/.axon_site/_ro/trn_rl_repo"):
    if _p not in sys.path:
        sys.path.insert(0, _p)

import bass_rust  # noqa: E402
from concourse import bass, bass_utils, mybir, tile  # noqa: E402

F32 = mybir.dt.float32
BF16 = mybir.dt.bfloat16
F16 = mybir.dt.float16
ALU = mybir.AluOpType
ACTF = mybir.ActivationFunctionType

B_FULL, S_FULL, D_FULL = 16, 2048, 256
N_CORES = 8


def split_multi_waits(nc, max_waits=1):
    """The walrus in this container encodes at most one sync-wait command per
    instruction. Hoist extra waits onto single-wait NoOps on the same engine
    right before the instruction — semantically identical (same blocking
    point in the engine's program order)."""
    n_split = 0
    counter = 0
    for func in nc.m.functions:
        for block in func.blocks:
            new = []
            for inst in list(block.instructions):
                si = getattr(inst, "sync_info", None)
                ow = list(si.on_wait) if si is not None and si.on_wait else []
                if len(ow) > max_waits:
                    for w in ow[:-max_waits]:
                        counter += 1
                        nop = bass_rust.InstNoOp(
                            name=f"WSPLIT-{counter}", ins=[], outs=[]
                        )
                        nop.engine = inst.engine
                        nop.sync_info = mybir.SyncInfo(on_wait=[w], on_update=[])
                        nop.debug = inst.debug
                        nc.register_instruction(nop, overwrite=True)
                        new.append(nop)
                    inst.sync_info = mybir.SyncInfo(
                        on_wait=ow[-max_waits:], on_update=list(si.on_update or [])
                    )
                    n_split += 1
                new.append(inst)
            block.instructions = new
    return n_split


def build_nc(BPC: int, S: int, D: int, rep: int = 1, no_transpose: bool = False, ablate: str = '') -> bass.Bass:
    """Build the per-core Bass module. BPC batches, sequence S, feature D.
    rep > 1 repeats the whole computation (for slope-based timing)."""
    NB = S // 128          # number of 128-row blocks
    KC = D // 128          # contraction chunks for QK (128 each)
    QN = min(S, 512)       # QK moving free dim (fp32 max 512)
    NQ = S // QN           # QK chunks per row-block
    SUB = 16               # stabilizer subsample stride

    nc = bass.Bass("TRN2", target_bir_lowering=False, debug=False)

    pmt_d = nc.dram_tensor("pmt", [BPC, KC, 128, S], F16, kind="ExternalInput").ap()
    pmbf_d = nc.dram_tensor("pmbf", [BPC, 128, NB * D], F16, kind="ExternalInput").ap()
    dep_d = nc.dram_tensor("dep", [BPC, S, S], F16, kind="ExternalInput").ap()
    negc_d = nc.dram_tensor("negc", [BPC, 128, NB], F32, kind="ExternalInput").ap()
    n0e_d = nc.dram_tensor("n0e", [BPC, 128, NB], F32, kind="ExternalInput").ap()
    pmv_d = nc.dram_tensor("pmv", [BPC, 128, NB], F32, kind="ExternalInput").ap()
    out_d = nc.dram_tensor("out", [BPC, S, D], F32, kind="ExternalOutput").ap()

    with ExitStack() as ctx:
        tc = ctx.enter_context(tile.TileContext(nc))
        const = ctx.enter_context(tc.tile_pool(name="const", bufs=1))
        batchp = ctx.enter_context(tc.tile_pool(name="batch", bufs=2))
        depp = ctx.enter_context(tc.tile_pool(name="depp", bufs=3))
        ework = ctx.enter_context(tc.tile_pool(name="ework", bufs=3))
        gwork = ctx.enter_context(tc.tile_pool(name="gwork", bufs=3))
        outp = ctx.enter_context(tc.tile_pool(name="outp", bufs=3))
        smalls = ctx.enter_context(tc.tile_pool(name="smalls", bufs=6))
        xp = ctx.enter_context(tc.tile_pool(name="xp", bufs=1, space="PSUM"))
        up = ctx.enter_context(tc.tile_pool(name="up", bufs=2, space="PSUM"))

        def emit_av(st):
            """Lagged second matmul + output scale/store for a finished block."""
            GT_, s_, pmbf_, b_, ib_ = st
            U = up.tile([128, D], F32, tag="U")
            for jc in range(NB):
                nc.tensor.matmul(
                    U[:],
                    lhsT=GT_[:, jc * 128:(jc + 1) * 128],
                    rhs=pmbf_[:, jc * D:(jc + 1) * D],
                    start=(jc == 0),
                    stop=(jc == NB - 1),
                )
            osb = outp.tile([128, D], F32, tag="osb")
            nc.vector.tensor_scalar_mul(osb[:], U[:], s_[:])
            nc.sync.dma_start(out_d[b_, ib_ * 128:(ib_ + 1) * 128, :], osb[:])

        pending = []
        for b in [bb for _ in range(rep) for bb in range(BPC)]:
            pmt_sb = batchp.tile([128, KC * S], F16, tag="pmt")
            for kc in range(KC):
                nc.sync.dma_start(pmt_sb[:, kc * S:(kc + 1) * S], pmt_d[b, kc])
            pmbf_sb = batchp.tile([128, NB * D], F16, tag="pmbf")
            nc.sync.dma_start(pmbf_sb[:], pmbf_d[b])
            negc_sb = batchp.tile([128, NB], F32, tag="negc")
            nc.sync.dma_start(negc_sb[:], negc_d[b])
            n0e_sb = batchp.tile([128, NB], F32, tag="n0e")
            nc.sync.dma_start(n0e_sb[:], n0e_d[b])
            pmv_sb = batchp.tile([128, NB], F32, tag="pmv")
            nc.sync.dma_start(pmv_sb[:], pmv_d[b])

            for ib in range(NB):
                dep_sb = depp.tile([128, S], F16, tag="dep")
                if ablate != "nodep":
                    nc.sync.dma_start(dep_sb[:], dep_d[b, ib * 128:(ib + 1) * 128, :])

                # x[i, j] = Pm_i . Pm_j  for this 128-row block, all S columns
                x = xp.tile([128, S], F32, tag="x")
                for kc in range(KC):
                    for jc in range(NQ):
                        nc.tensor.matmul(
                            x[:, jc * QN:(jc + 1) * QN],
                            lhsT=pmt_sb[:, kc * S + ib * 128: kc * S + ib * 128 + 128],
                            rhs=pmt_sb[:, kc * S + jc * QN: kc * S + (jc + 1) * QN],
                            start=(kc == 0),
                            stop=(kc == KC - 1),
                            skip_group_check=True,
                        )

                # E = exp(x - c), Z = row sum of E (fused); bias from host
                E = ework.tile([128, S], F32, tag="E")
                Z = smalls.tile([128, 1], F32, tag="Z")
                nc.scalar.activation(
                    E[:], x[:], ACTF.Exp, bias=negc_sb[:, ib:ib + 1],
                    scale=1.0, accum_out=Z[:]
                )

                # D1 = Z - n0*exp(-c)  (the 1e-13*Z term is below f32 ulp)
                d1 = smalls.tile([128, 1], F32, tag="d1")
                nc.vector.scalar_tensor_tensor(
                    d1[:], Z[:], 1.0 + 1e-13, n0e_sb[:, ib:ib + 1],
                    op0=ALU.mult, op1=ALU.subtract
                )
                rec = smalls.tile([128, 1], F32, tag="rec")
                nc.vector.reciprocal(rec[:], d1[:])
                s = smalls.tile([128, 1], F32, tag="s")
                nc.vector.tensor_scalar_mul(s[:], rec[:], pmv_sb[:, ib:ib + 1])

                if ablate in ("qkonly", "nodep"):
                    xr = smalls.tile([128, 1], F32, tag="xr")
                    nc.vector.tensor_copy(xr[:], x[:, 0:1])
                    osb0 = outp.tile([128, D], F32, tag="osb")
                    nc.vector.tensor_scalar_mul(osb0[:], pmt_sb[:, 0:D], xr[:])
                    nc.sync.dma_start(out_d[b, ib * 128:(ib + 1) * 128, :], osb0[:])
                    continue
                if ablate == "front":
                    # keep only QK/exp/stats + dep load + a tiny store
                    osb0 = outp.tile([128, D], F32, tag="osb")
                    nc.vector.tensor_scalar_mul(osb0[:], E[:, 0:D], s[:])
                    nc.sync.dma_start(out_d[b, ib * 128:(ib + 1) * 128, :], osb0[:])
                    continue
                # G = E * dep: halves on Pool and DVE in parallel
                G = gwork.tile([128, S], F16, tag="G")
                h = S // 2
                nc.gpsimd.tensor_tensor(G[:, 0:h], E[:, 0:h], dep_sb[:, 0:h],
                                        op=ALU.mult)
                nc.vector.tensor_tensor(G[:, h:S], E[:, h:S], dep_sb[:, h:S],
                                        op=ALU.mult)

                # GT[j, i] = G[i, j]: all NB 128x128 tiles transposed in one
                # xbar DMA (out AP places transposed blocks side by side)
                GT = gwork.tile([128, S], F16, tag="GT")
                gt_view = GT[:].rearrange("p (jc i) -> p jc i", i=128)
                if no_transpose:
                    nc.scalar.dma_start(GT[:], G[:])  # timing probe only
                else:
                    nc.scalar.dma_start(gt_view, G[:], transpose=True)

                # AV lags two blocks: keeps PE busy while this block's
                # softmax chain (exp -> G -> GT transpose) completes off-PE.
                pending.append((GT, s, pmbf_sb, b, ib))
                if len(pending) > 2:
                    emit_av(pending.pop(0))

        for st in pending:
            emit_av(st)
        del pending[:]

    split_multi_waits(nc)
    return nc


def make_in_maps(premise_batch, premise_mask, dependency_mask, n_cores=N_CORES):
    """Host-side preprocessing + sharding. Returns (in_maps, BPC)."""
    B, S, D = premise_batch.shape
    BPC = B // n_cores
    NB = S // 128
    KC = D // 128

    m = np.asarray(premise_mask).astype(np.float32)                    # [B,S]
    Pm = np.asarray(premise_batch).astype(np.float32) * m[:, :, None]  # [B,S,D]
    PmT = np.ascontiguousarray(Pm.transpose(0, 2, 1)).astype(np.float16).reshape(B, KC, 128, S)
    Pm_bf = np.ascontiguousarray(
        Pm.astype(np.float16).reshape(B, NB, 128, D).transpose(0, 2, 1, 3)
    ).reshape(B, 128, NB * D)
    dep_bf = np.asarray(dependency_mask).astype(np.float16)    # [B,S,S]
    diag = np.einsum("bsd,bsd->bs", Pm, Pm).astype(np.float32)
    negc = -np.maximum(diag, 0.0).astype(np.float32)                   # [B,S]
    n0 = (S - m.sum(axis=1)).astype(np.float32)                        # [B]
    n0e = (n0[:, None] * np.exp(negc)).astype(np.float32)              # [B,S]
    negc_r = np.ascontiguousarray(negc.reshape(B, NB, 128).transpose(0, 2, 1))
    n0e_r = np.ascontiguousarray(n0e.reshape(B, NB, 128).transpose(0, 2, 1))
    pmv_r = np.ascontiguousarray(m.reshape(B, NB, 128).transpose(0, 2, 1))

    in_maps = []
    for k in range(n_cores):
        sl = slice(k * BPC, (k + 1) * BPC)
        in_maps.append(
            {
                "pmt": np.ascontiguousarray(PmT[sl]),
                "pmbf": np.ascontiguousarray(Pm_bf[sl]),
                "dep": np.ascontiguousarray(dep_bf[sl]),
                "negc": np.ascontiguousarray(negc_r[sl]),
                "n0e": np.ascontiguousarray(n0e_r[sl]),
                "pmv": np.ascontiguousarray(pmv_r[sl]),
            }
        )
    return in_maps, BPC


_CACHE: dict = {}


def _built_nc() -> bass.Bass:
    if "nc" not in _CACHE:
        _CACHE["nc"] = build_nc(B_FULL // N_CORES, S_FULL, D_FULL)
    return _CACHE["nc"]


def kernel(premise_batch, premise_mask, dependency_mask, **run_kwargs):
    in_maps, _ = make_in_maps(premise_batch, premise_mask, dependency_mask)
    nc = _built_nc()
    res = bass_utils.run_bass_kernel_spmd(
        nc, in_maps, list(range(N_CORES)), **run_kwargs
    )
    outs = [np.asarray(res.results[k]["out"]) for k in range(N_CORES)]
    full = np.concatenate(outs, axis=0).astype(np.float32)
    if run_kwargs:
        _CACHE["last_results"] = res
    return full
